# revision 10
# baseline (speedup 1.0000x reference)
"""GCN encoder kernel for Trainium2, SPMD across 8 NeuronCores.

Computes (reference semantics):
    x_ = P @ (x @ W1 + b1)
    h  = P @ (1.8 * l2norm_rows(x @ W2 + b2))
where P = D^-1/2 (A + I) D^-1/2 over the edge list (by destination).

Strategy (v2):
  * Both branches concatenated into one feature matrix u[N, 256] (bf16),
    pre-scaled by dinv[src]; computed replicated on every core (phase A).
    u rows are stored in a core-interleaved permutation pi(n) =
    (n % NPC)*8 + n // NPC so that every window's self-loop sources fall
    in the same source segment on all cores.
  * Bias is injected into PSUM via a 1-partition matmul (ones x [b1|b2]),
    the norm uses a fused Rsqrt activation.
  * Edges (incl. self loops) partitioned by destination across 8 cores,
    grouped into (pair-of-128-dst-windows x source-segment) chunks and
    processed SEGMENT-MAJOR so gathers of segment s start as soon as
    phase A has produced u rows of segment s.  Window partial sums are
    accumulated in an SBUF-resident f32 accumulator (one [128,256] slab
    per window), PSUM holds only one (pair,seg) chunk at a time.
  * Per chunk: ONE indirect-DMA gather (idx-0 padding equalizes the
    per-core valid count; -1 sentinel indices generate no descriptors),
    ONE broadcast-compare DVE op builds all one-hot S tiles, then a
    PSUM matmul chain per window half.
"""
import sys

import numpy as np

try:
    import concourse.bass as bass  # noqa: F401
except ImportError:
    sys.path.insert(0, "/opt/trn_rl_repo")

from contextlib import ExitStack

from ml_dtypes import bfloat16

import concourse.bass as bass
import concourse.bacc as bacc
import concourse.tile as tile
from concourse import mybir
from concourse.bass_utils import run_bass_kernel_spmd

N_CORES = 8
WIN = 128
PAIR_W = 2  # windows per gather chunk
ROWG = 512  # phase-A row group (per xT load)
NSEG = 4    # u-row segments for int16 dma_gather indices (seg size < 32768)
MSG_BUFS = 4
S_BUFS = 4


def _dims(n_nodes, c_in, c_out):
    npc = n_nodes // N_CORES
    nwin = -(-npc // WIN)
    npair = -(-nwin // PAIR_W)
    u_rows = -(-n_nodes // ROWG) * ROWG
    seg = u_rows // NSEG
    assert seg % 128 == 0 and seg <= 32768
    return dict(
        N=n_nodes, CIN=c_in, COUT=c_out, C=2 * c_out, NPC=npc, NWIN=nwin,
        NPAIR=npair, NCH=NSEG * npair, U_ROWS=u_rows, G=u_rows // ROWG,
        KCH=c_in // 128, SEG=seg,
    )


def _prep(x, edge_index, W1, b1, W2, b2):
    """Host-side sharding: degrees, edge partitioning/packing, dtype prep."""
    x = np.asarray(x, np.float32)
    n, c_in = x.shape
    c_out = W1.shape[1]
    d = _dims(n, c_in, c_out)
    NPC, NPAIR, SEG = d["NPC"], d["NPAIR"], d["SEG"]

    src = np.asarray(edge_index[0], np.int64)
    dst = np.asarray(edge_index[1], np.int64)

    deg = (np.bincount(dst, minlength=n) + 1).astype(np.float32)
    dinv = (1.0 / np.sqrt(deg)).astype(np.float32)

    # combined edge list: real edges + self loops
    n_all = np.arange(n, dtype=np.int64)
    src_a = np.concatenate([src, n_all])
    dst_a = np.concatenate([dst, n_all])
    core = dst_a // NPC
    rem = dst_a % NPC
    win = rem // WIN
    slot = (rem % WIN).astype(np.float32)
    pair = win // PAIR_W
    wi = win % PAIR_W
    # pi-permuted source row (core-interleaved) and its segment
    row = (src_a % NPC) * N_CORES + src_a // NPC
    seg = row // SEG
    gk = (((core * NPAIR + pair) * NSEG + seg) * PAIR_W + wi)
    order = np.lexsort((row, gk))
    row_o = row[order]
    gk_o = gk[order]
    seg_o = seg[order]
    core_o = core[order]
    pair_o = pair[order]
    wi_o = wi[order]
    slot_o = slot[order]

    ngrp = N_CORES * NPAIR * NSEG * PAIR_W
    cnt = np.bincount(gk_o, minlength=ngrp).reshape(
        N_CORES, NPAIR, NSEG, PAIR_W)
    cap = np.maximum(-(-cnt.max(axis=0) // 128), 1)      # [NPAIR, NSEG, 2]
    V = np.maximum(cnt.sum(axis=3).max(axis=0), 1)       # [NPAIR, NSEG]
    Tps = cap.sum(axis=2)                                # [NPAIR, NSEG]
    capw = int(Tps.max())
    d["CAP"] = cap
    d["V"] = V
    d["TPS"] = Tps
    d["CAPW"] = capw

    # position within each (core,pair,seg,wi) group
    start = np.zeros(ngrp, np.int64)
    csum = np.cumsum(cnt.reshape(-1))
    start[1:] = csum[:-1]
    pos = np.arange(row_o.shape[0], dtype=np.int64) - start[gk_o]
    # logical index within the chunk's tile stream (w1 starts at cap0*128)
    li = pos + wi_o * cap[pair_o, seg_o, 0] * 128
    chunk = seg_o * NPAIR + pair_o  # emission order: seg-major

    NCH = d["NCH"]
    neg1_bf16 = np.float32(-1.0).astype(bfloat16).view(np.int16)
    # all positions hold a valid index (0 = first row of the segment);
    # padding entries carry slot -1 so their S column is all-zero.
    eidx = np.zeros((N_CORES, NCH, 16, capw * 8), np.int16)
    es16 = np.full((N_CORES, NCH, 128, capw), neg1_bf16, np.int16)
    eidx[core_o, chunk, li % 16, li // 16] = (row_o - seg_o * SEG).astype(
        np.int16)
    es16[core_o, chunk, li % 128, li // 128] = slot_o.astype(bfloat16).view(
        np.int16)
    eidx = np.tile(eidx, (1, 1, 8, 1))  # replicate 16 -> 128 partitions
    edi_buf = np.concatenate([eidx, es16], axis=3)  # [8, NCH, 128, capw*9]

    # pi-permuted per-row tables
    pi = (n_all % NPC) * N_CORES + n_all // NPC
    dinv_pad = np.ones(d["U_ROWS"], np.float32)
    dinv_pad[pi] = dinv
    dinvr = np.ascontiguousarray(dinv_pad.reshape(-1, 128).T)  # [128, nrt]

    dd = np.ones((N_CORES, d["NWIN"] * WIN), np.float32)
    dd[:, :NPC] = dinv.reshape(N_CORES, NPC)
    dinvd = np.ascontiguousarray(
        dd.reshape(N_CORES, d["NWIN"], WIN).transpose(0, 2, 1))  # [8,128,NWIN]

    xt = np.zeros((c_in, d["U_ROWS"]), bfloat16)
    xt[:, pi] = x.T.astype(bfloat16)
    wc = np.concatenate([W1, W2], axis=1).astype(bfloat16)  # [CIN, 2*COUT]
    biasr = np.concatenate([b1, b2]).astype(bfloat16)[None, :]  # [1, C]
    iota_bc = np.tile(
        np.arange(128, dtype=np.float32)[None, :], (128, 1)).astype(bfloat16)

    in_maps = []
    for k in range(N_CORES):
        in_maps.append({
            "xt": xt,
            "wc": wc,
            "biasr": biasr,
            "iota": iota_bc,
            "dinvr": dinvr,
            "dinvd": dinvd[k],
            "edi": edi_buf[k],
        })
    return in_maps, d


def _build(d):
    """Emit the SPMD Bass program (identical on all cores; data differs)."""
    f32, bf16 = mybir.dt.float32, mybir.dt.bfloat16
    i16 = mybir.dt.int16
    C, CIN, KCH = d["C"], d["CIN"], d["KCH"]
    SEG, NPAIR, NWIN = d["SEG"], d["NPAIR"], d["NWIN"]
    CAP, V, TPS, CAPW = d["CAP"], d["V"], d["TPS"], d["CAPW"]
    nrt = d["U_ROWS"] // 128
    co = d["COUT"]
    inv_s2 = 1.0 / (1.8 * 1.8)

    nc = bacc.Bacc("TRN2", target_bir_lowering=False, debug=False,
                   num_swdge_queues=4)
    xt_d = nc.dram_tensor("xt", [CIN, d["U_ROWS"]], bf16, kind="ExternalInput")
    wc_d = nc.dram_tensor("wc", [CIN, C], bf16, kind="ExternalInput")
    biasr_d = nc.dram_tensor("biasr", [1, C], bf16, kind="ExternalInput")
    iota_d = nc.dram_tensor("iota", [128, 128], bf16, kind="ExternalInput")
    dinvr_d = nc.dram_tensor("dinvr", [128, nrt], f32, kind="ExternalInput")
    dinvd_d = nc.dram_tensor("dinvd", [128, NWIN], f32, kind="ExternalInput")
    edi_d = nc.dram_tensor("edi", [d["NCH"], 128, CAPW * 9], i16,
                           kind="ExternalInput")
    out_d = nc.dram_tensor("out", [d["NPC"], C], f32, kind="ExternalOutput")
    u_d = nc.dram_tensor("u", [d["U_ROWS"], C], bf16)  # internal scratch

    with ExitStack() as ctx:
        tc = ctx.enter_context(tile.TileContext(nc))
        const_p = ctx.enter_context(tc.tile_pool(name="const", bufs=1))
        xa_p = ctx.enter_context(tc.tile_pool(name="xa", bufs=4))
        sq_p = ctx.enter_context(tc.tile_pool(name="sq", bufs=4))
        col_p = ctx.enter_context(tc.tile_pool(name="col", bufs=12))
        ua_p = ctx.enter_context(tc.tile_pool(name="ua", bufs=6))
        ed_p = ctx.enter_context(tc.tile_pool(name="ed", bufs=4))
        msg_p = ctx.enter_context(tc.tile_pool(name="msg", bufs=MSG_BUFS))
        s_p = ctx.enter_context(tc.tile_pool(name="s", bufs=S_BUFS))
        acc_p = ctx.enter_context(tc.tile_pool(name="acc", bufs=1))
        out_p = ctx.enter_context(tc.tile_pool(name="o", bufs=3))
        psa_p = ctx.enter_context(tc.tile_pool(name="psa", bufs=4, space="PSUM"))
        psb_p = ctx.enter_context(tc.tile_pool(name="psb", bufs=4, space="PSUM"))

        # constants
        wc_t = [const_p.tile([128, C], bf16, name=f"wct{kc}", tag=f"wc{kc}")
                for kc in range(KCH)]
        for kc in range(KCH):
            nc.sync.dma_start(out=wc_t[kc][:], in_=wc_d[kc * 128:(kc + 1) * 128, :])
        biasr_t = const_p.tile([1, C], bf16)
        nc.sync.dma_start(out=biasr_t[:], in_=biasr_d[:, :])
        ones_t = const_p.tile([1, 128], bf16)
        nc.vector.memset(ones_t[:], 1.0)
        iota_t = const_p.tile([128, 128], bf16)
        nc.sync.dma_start(out=iota_t[:], in_=iota_d[:, :])
        dinvr_t = const_p.tile([128, nrt], f32)
        nc.sync.dma_start(out=dinvr_t[:], in_=dinvr_d[:, :])
        dinvd_t = const_p.tile([128, NWIN], f32)
        nc.sync.dma_start(out=dinvd_t[:], in_=dinvd_d[:, :])
        eps_t = const_p.tile([128, 1], f32)
        nc.vector.memset(eps_t[:], 1e-24)
        zeros_t = const_p.tile([128, C], f32)
        nc.vector.memset(zeros_t[:], 0.0)

        # window accumulators: one [128, C] f32 slab per window
        acc_t = acc_p.tile([128, NWIN * C], f32)

        # ---- phase A: u[r] = [dinv*(x@W1+b1) | dinv*1.8*l2n(x@W2+b2)] ----
        for g in range(d["G"]):
            xg = [xa_p.tile([128, ROWG], bf16, name=f"xg{kc}", tag=f"xg{kc}")
                  for kc in range(KCH)]
            for kc in range(KCH):
                nc.sync.dma_start(
                    out=xg[kc][:],
                    in_=xt_d[kc * 128:(kc + 1) * 128, g * ROWG:(g + 1) * ROWG])
            for jj in range(ROWG // 128):
                rt = g * (ROWG // 128) + jj
                ps = psa_p.tile([128, C], f32)
                nc.tensor.matmul(ps[:], lhsT=ones_t[:1, :], rhs=biasr_t[:1, :],
                                 start=True, stop=False)
                for kc in range(KCH):
                    nc.tensor.matmul(
                        ps[:], lhsT=xg[kc][:, jj * 128:(jj + 1) * 128],
                        rhs=wc_t[kc][:], start=False, stop=(kc == KCH - 1))
                u_t = ua_p.tile([128, C], bf16)
                # branch-2 eviction (unscaled) to SBUF bf16, then norm stats
                nc.scalar.activation(
                    out=u_t[:, co:], in_=ps[:, co:],
                    func=mybir.ActivationFunctionType.Copy,
                    bias=0.0, scale=1.0)
                sq_t = sq_p.tile([128, co], bf16)
                s_col = col_p.tile([128, 1], f32, tag="scol")
                nc.vector.scalar_tensor_tensor(
                    out=sq_t[:], in0=u_t[:, co:], scalar=1.0, in1=u_t[:, co:],
                    op0=mybir.AluOpType.mult, op1=mybir.AluOpType.mult,
                    accum_out=s_col[:])
                nrm = col_p.tile([128, 1], f32, tag="nrm")
                nc.scalar.activation(
                    out=nrm[:], in_=s_col[:],
                    func=mybir.ActivationFunctionType.Sqrt,
                    bias=eps_t[:], scale=inv_s2)
                rn = col_p.tile([128, 1], f32, tag="rn")
                nc.vector.reciprocal(out=rn[:], in_=nrm[:])
                phi2 = col_p.tile([128, 1], f32, tag="phi2")
                nc.vector.tensor_tensor(
                    out=phi2[:], in0=rn[:], in1=dinvr_t[:, rt:rt + 1],
                    op=mybir.AluOpType.mult)
                # in-place branch-2 scale on DVE (bf16)
                nc.vector.scalar_tensor_tensor(
                    out=u_t[:, co:], in0=u_t[:, co:], scalar=phi2[:],
                    in1=zeros_t[:, :co],
                    op0=mybir.AluOpType.mult, op1=mybir.AluOpType.add)
                if rt % 2 != 0:  # DVE and ACT split the u1 evictions
                    nc.vector.scalar_tensor_tensor(
                        out=u_t[:, :co], in0=ps[:, :co],
                        scalar=dinvr_t[:, rt:rt + 1], in1=zeros_t[:, :co],
                        op0=mybir.AluOpType.mult, op1=mybir.AluOpType.add)
                else:
                    nc.scalar.activation(
                        out=u_t[:, :co], in_=ps[:, :co],
                        func=mybir.ActivationFunctionType.Copy,
                        bias=0.0, scale=dinvr_t[:, rt:rt + 1])
                nc.sync.dma_start(
                    out=u_d[rt * 128:(rt + 1) * 128, :], in_=u_t[:])

        # ---- phase B: seg-major chunked gather + segment matmul ----
        for s in range(NSEG):
            for p in range(NPAIR):
                ci = s * NPAIR + p
                tps = int(TPS[p, s])
                vv = int(V[p, s])
                ei_t = ed_p.tile([128, CAPW * 9], i16, tag="ei")
                nc.sync.dma_start(out=ei_t[:], in_=edi_d[ci, :, :])
                msg_t = msg_p.tile([128, CAPW, C], bf16)
                nc.gpsimd.dma_gather(
                    out_ap=msg_t[:, :tps, :],
                    in_ap=u_d[s * SEG:(s + 1) * SEG, :],
                    idxs_ap=ei_t[:, :tps * 8],
                    num_idxs=tps * 128,
                    num_idxs_reg=tps * 128,
                    elem_size=C,
                    single_packet=False,
                    queue_num=ci % 4)
                # one DVE op builds all one-hot S tiles of the chunk
                s_t = s_p.tile([128, CAPW * 128], bf16)
                es_ap = ei_t[:, CAPW * 8:CAPW * 8 + tps].bitcast(bf16)
                in0 = es_ap.unsqueeze(2).broadcast_to([128, tps, 128])
                in1 = iota_t[:].unsqueeze(1).broadcast_to([128, tps, 128])
                s_ap = s_t[:, :tps * 128].rearrange("p (t q) -> p t q", t=tps)
                nc.vector.tensor_tensor(
                    out=s_ap, in0=in0, in1=in1, op=mybir.AluOpType.is_equal)
                # matmul chains: one [128, 2C] PSUM bank for the pair
                ps = psb_p.tile([128, 2 * C], f32)
                t0 = 0
                for wi in range(PAIR_W):
                    w = p * PAIR_W + wi
                    if w >= NWIN:
                        break
                    ncap = int(CAP[p, s, wi])
                    for k in range(ncap):
                        t = t0 + k
                        nc.tensor.matmul(
                            ps[:, wi * C:(wi + 1) * C],
                            lhsT=s_t[:, t * 128:(t + 1) * 128],
                            rhs=msg_t[:, t, :],
                            start=(k == 0), stop=(k == ncap - 1))
                    t0 += ncap
                nw = min(NWIN - p * PAIR_W, PAIR_W)
                a_ap = acc_t[:, p * PAIR_W * C:(p * PAIR_W + nw) * C]
                if s == 0:
                    nc.scalar.activation(
                        out=a_ap, in_=ps[:, :nw * C],
                        func=mybir.ActivationFunctionType.Copy,
                        bias=0.0, scale=1.0)
                else:
                    nc.vector.tensor_tensor(
                        out=a_ap, in0=a_ap, in1=ps[:, :nw * C],
                        op=mybir.AluOpType.add)
                if s == NSEG - 1:
                    for wi in range(nw):
                        w = p * PAIR_W + wi
                        o_t = out_p.tile([128, C], f32)
                        nc.scalar.activation(
                            out=o_t[:], in_=acc_t[:, w * C:(w + 1) * C],
                            func=mybir.ActivationFunctionType.Copy,
                            bias=0.0, scale=dinvd_t[:, w:w + 1])
                        rows = min(d["NPC"] - w * 128, 128)
                        nc.sync.dma_start(
                            out=out_d[w * 128:w * 128 + rows, :],
                            in_=o_t[:rows, :])

    nc.compile()
    return nc


def _run(in_maps, d, trace=False):
    nc = _build(d)
    res = run_bass_kernel_spmd(
        nc, in_maps, core_ids=list(range(N_CORES)), trace=trace)
    outs = np.concatenate(
        [res.results[k]["out"] for k in range(N_CORES)], axis=0)
    co = d["COUT"]
    x_ = np.ascontiguousarray(outs[:, :co])
    h = np.ascontiguousarray(outs[:, co:])
    return (h, x_), res


def kernel(x, edge_index, W1, b1, W2, b2):
    in_maps, d = _prep(x, edge_index, W1, b1, W2, b2)
    (h, x_), _ = _run(in_maps, d, trace=False)
    return (h, x_)
